# revision 1
# baseline (speedup 1.0000x reference)
"""BCQConv1D TRN2 kernel: out[b,s,o] = x[b,s,:] @ W[o,:]^T + bias[o],
W[o, g*A+a] = sum_qb alpha[o,g,qb] * binary[o,g,a,qb].

Sharding: column-parallel — alpha/binary/bias split along out_features
across the 8 NeuronCores, x replicated (each core computes the full
batch for its 512 output features).

Per core: reconstruct the W^T shard on device (DVE fused mul-add over
the 3 BCQ bit planes + PE transpose), keep it resident in SBUF as
float32r, then stream x^T tiles and run N=512 float32r matmuls (full
bf16 rate on the PE, ~13 effective mantissa bits) accumulating over
K=4096 in PSUM. Bias is broadcast once via a K=1 ones-matmul and folded
into the PSUM->SBUF output add on the DVE.

Host side only slices/relayouts inputs (x is passed transposed/tiled
[128, KT, BS] so every DMA line is 2KB contiguous).
"""

import numpy as np

import concourse.bass as bass
import concourse.tile as tile
from concourse import bacc, mybir
from concourse.bass_utils import run_bass_kernel_spmd
from concourse.masks import make_identity

# Problem shape (hardcoded per contest contract)
B, S, I, O = 4, 2048, 4096, 4096
G, A, QB = 32, 128, 3
BS = B * S  # 8192
P = 128
KT = I // P  # 32 k-tiles (== groups: i = g*A + a, A == P)

# Sharding
N_CORES = 8
O_WAYS = 8
BS_WAYS = 1
O_SH = O // O_WAYS  # per-core out features
BS_SH = BS // BS_WAYS  # per-core batch rows
NFREE = 512  # matmul moving free dim (one PSUM bank of fp32)
NB = O_SH // NFREE  # o-blocks per core
BCHUNK = 512  # bs columns fetched per x DMA
NSUB = BCHUNK // P  # matmul chains per x chunk
GMERGE = 2  # binary groups per recon DMA

F32 = mybir.dt.float32
F32R = mybir.dt.float32r


def build_nc():
    nc = bacc.Bacc(target_bir_lowering=False)
    xt_d = nc.declare_dram_parameter("xt", [P, KT, BS_SH], F32R, isOutput=False)
    alpha_d = nc.declare_dram_parameter("alpha", [O_SH, G, QB], F32, isOutput=False)
    binary_d = nc.declare_dram_parameter("binary", [O_SH, G, A, QB], F32, isOutput=False)
    bias_d = nc.declare_dram_parameter("bias", [O_SH], F32, isOutput=False)
    out_d = nc.declare_dram_parameter("out", [BS_SH, O_SH], F32, isOutput=True)

    OT = O_SH // P  # o-tiles for recon
    add = mybir.AluOpType.add
    mult = mybir.AluOpType.mult

    with tile.TileContext(nc) as tc:
        with (
            tc.tile_pool(name="const", bufs=1) as cpool,
            tc.tile_pool(name="wt", bufs=1) as wtpool,
            tc.tile_pool(name="rec", bufs=4) as rec,
            tc.tile_pool(name="wog", bufs=4) as wog_pool,
            tc.tile_pool(name="xp", bufs=10) as xp,
            tc.tile_pool(name="op", bufs=8) as op,
            tc.tile_pool(name="psum", bufs=8, space="PSUM") as pp,
        ):
            # --- constants ---
            ident = cpool.tile([P, P], F32, name="ident")
            make_identity(nc, ident)
            ones = cpool.tile([1, P], F32, name="ones")
            nc.vector.memset(ones, 1.0)
            bias_row = cpool.tile([1, O_SH], F32, name="bias_row")
            nc.sync.dma_start(out=bias_row, in_=bias_d.ap().unsqueeze(0))
            bias_bc = cpool.tile([P, O_SH], F32, name="bias_bc")
            for j in range(NB):
                pbt = pp.tile([P, NFREE], F32, tag="ps", name=f"psb{j}")
                nc.tensor.matmul(
                    pbt, ones, bias_row[:, j * NFREE : (j + 1) * NFREE],
                    start=True, stop=True,
                )
                nc.vector.tensor_copy(
                    out=bias_bc[:, j * NFREE : (j + 1) * NFREE], in_=pbt
                )

            # --- alpha (per-partition scalars), all o-tiles resident ---
            alpha_sb = []
            for ot in range(OT):
                at = cpool.tile([P, G, QB], F32, name=f"alpha{ot}")
                nc.sync.dma_start(out=at, in_=alpha_d.ap()[ot * P : (ot + 1) * P])
                alpha_sb.append(at)

            # --- W^T shard, resident, one tile per k-tile (== group) ---
            wt_tiles = [
                wtpool.tile([P, O_SH], F32R, tag=f"wt{k}", name=f"wt{k}")
                for k in range(KT)
            ]

            # --- reconstruction: W[o, g*A + a] then PE-transpose to W^T ---
            for gp in range(G // GMERGE):
                for ot in range(OT):
                    bt = rec.tile([P, GMERGE, A, QB], F32, tag="bt")
                    nc.sync.dma_start(
                        out=bt,
                        in_=binary_d.ap()[
                            ot * P : (ot + 1) * P,
                            gp * GMERGE : (gp + 1) * GMERGE,
                        ],
                    )
                    at = alpha_sb[ot]
                    for gg in range(GMERGE):
                        g = gp * GMERGE + gg
                        w_og = wog_pool.tile([P, P], F32, tag="wog")
                        nc.vector.tensor_scalar_mul(
                            w_og, bt[:, gg, :, 0], at[:, g, 0:1]
                        )
                        nc.vector.scalar_tensor_tensor(
                            w_og, bt[:, gg, :, 1], at[:, g, 1:2], w_og, mult, add
                        )
                        nc.vector.scalar_tensor_tensor(
                            w_og, bt[:, gg, :, 2], at[:, g, 2:3], w_og, mult, add
                        )
                        ptt = pp.tile([P, P], F32, tag="ps", name=f"ptr{g}_{ot}")
                        nc.tensor.transpose(ptt, w_og, ident)
                        nc.vector.tensor_copy(
                            out=wt_tiles[g][:, ot * P : (ot + 1) * P], in_=ptt
                        )

            # --- main matmul: out[bs, o] = x^T.T @ W^T (+bias) ---
            n_chunks = BS_SH // BCHUNK
            for c in range(n_chunks):
                psums = [
                    [
                        pp.tile([P, NFREE], F32, tag="ps", name=f"mm{c}_{s}_{j}")
                        for j in range(NB)
                    ]
                    for s in range(NSUB)
                ]
                for k in range(KT):
                    xt_t = xp.tile([P, BCHUNK], F32R, tag="xt")
                    dma_eng = nc.sync if k % 2 == 0 else nc.scalar
                    dma_eng.dma_start(
                        out=xt_t,
                        in_=xt_d.ap()[:, k, c * BCHUNK : (c + 1) * BCHUNK],
                    )
                    for s in range(NSUB):
                        for j in range(NB):
                            nc.tensor.matmul(
                                psums[s][j],
                                xt_t[:, s * P : (s + 1) * P],
                                wt_tiles[k][:, j * NFREE : (j + 1) * NFREE],
                                start=(k == 0),
                                stop=(k == KT - 1),
                            )
                for s in range(NSUB):
                    for j in range(NB):
                        os_t = op.tile([P, NFREE], F32, tag="os")
                        nc.vector.tensor_tensor(
                            out=os_t,
                            in0=psums[s][j],
                            in1=bias_bc[:, j * NFREE : (j + 1) * NFREE],
                            op=add,
                        )
                        nc.sync.dma_start(
                            out=out_d.ap()[
                                c * BCHUNK + s * P : c * BCHUNK + (s + 1) * P,
                                j * NFREE : (j + 1) * NFREE,
                            ],
                            in_=os_t,
                        )

    if not nc.is_finalized():
        nc.finalize()
    return nc


def shard_inputs(x, alpha, bias, binary):
    """Host-side slicing/relayout only. Returns per-core input maps."""
    x2 = np.ascontiguousarray(x).reshape(BS, I)
    # xtp[p, k, s] = x2[s, k*P + p]  -> every DMA line is bs-contiguous
    xtp = np.ascontiguousarray(x2.T.reshape(KT, P, BS).transpose(1, 0, 2))
    alpha = np.ascontiguousarray(alpha)
    binary = np.ascontiguousarray(binary)
    bias = np.ascontiguousarray(bias)

    in_maps = []
    for c in range(N_CORES):
        oc, bc = divmod(c, BS_WAYS)
        osl = slice(oc * O_SH, (oc + 1) * O_SH)
        if BS_WAYS == 1:
            xc = xtp
        else:
            xc = np.ascontiguousarray(xtp[:, :, bc * BS_SH : (bc + 1) * BS_SH])
        in_maps.append(
            {
                "xt": xc,
                "alpha": alpha[osl],
                "binary": binary[osl],
                "bias": bias[osl],
            }
        )
    return in_maps


def assemble_output(results):
    out = np.empty((BS, O), dtype=np.float32)
    for c in range(N_CORES):
        oc, bc = divmod(c, BS_WAYS)
        out[
            bc * BS_SH : (bc + 1) * BS_SH, oc * O_SH : (oc + 1) * O_SH
        ] = results[c]["out"]
    return out.reshape(B, S, O)


_NC_CACHE = None


def kernel(x, alpha, bias, binary):
    global _NC_CACHE
    if _NC_CACHE is None:
        _NC_CACHE = build_nc()
    nc = _NC_CACHE
    in_maps = shard_inputs(
        np.asarray(x, dtype=np.float32),
        np.asarray(alpha, dtype=np.float32),
        np.asarray(bias, dtype=np.float32),
        np.asarray(binary, dtype=np.float32),
    )
    res = run_bass_kernel_spmd(nc, in_maps, list(range(N_CORES)))
    return assemble_output(res.results)



# revision 3
# speedup vs baseline: 1.8534x; 1.8534x over previous
"""BCQConv1D TRN2 kernel v8.

v3b + recon/main-loop FUSION: the PE engine queue is FIFO, so v3b's 128
recon transposes blocked every main matmul until recon finished. Here the
emission interleaves: per bit-plane group g, emit recon(g) then chunk 0's
and chunk 1's k=g matmul units (their accumulation consumes wt[g] exactly
as recon produces it). Chunk 1's s2/s3 units are deferred past the fused
loop (only 6 main PSUM banks: 4 for chunk 0 + 2 for chunk 1; recon ptt
tiles own the other 2 banks). Recon copies are split ACT/DVE to balance
engine time.

Datapath as v3b: bf16 x / W^T, fp8e4 binary, fp32 alpha/bias/psum,
split-bank N=256 matmul pairs, bf16 out + host upcast.
"""

import numpy as np
import ml_dtypes

import concourse.bass as bass
import concourse.tile as tile
from concourse import bacc, mybir
from concourse.bass_utils import run_bass_kernel_spmd
from concourse.masks import make_identity

B, S, I, O = 4, 2048, 4096, 4096
G, A, QB = 32, 128, 3
BS = B * S
P = 128
KT = I // P

N_CORES = 8
O_WAYS = 8
BS_WAYS = 1
O_SH = O // O_WAYS
BS_SH = BS // BS_WAYS
NFREE = 512
BCHUNK = 512
NSUB = BCHUNK // P
GMERGE = 4

F32 = mybir.dt.float32
BF16 = mybir.dt.bfloat16
FP8 = mybir.dt.float8e4
NP_BF16 = ml_dtypes.bfloat16
NP_FP8 = ml_dtypes.float8_e4m3fn


def build_nc():
    nc = bacc.Bacc(target_bir_lowering=False)
    xt_d = nc.declare_dram_parameter("xt", [P, KT, BS_SH], BF16, isOutput=False)
    alpha_d = nc.declare_dram_parameter("alpha", [O_SH, G, QB], F32, isOutput=False)
    binary_d = nc.declare_dram_parameter(
        "binary", [O_SH, G, A, QB], FP8, isOutput=False
    )
    bias_d = nc.declare_dram_parameter("bias", [O_SH], F32, isOutput=False)
    out_d = nc.declare_dram_parameter("out", [BS_SH, O_SH], BF16, isOutput=True)

    OT = O_SH // P
    add = mybir.AluOpType.add
    mult = mybir.AluOpType.mult
    H = NFREE // 2

    with tile.TileContext(nc) as tc:
        with (
            tc.tile_pool(name="const", bufs=1) as cpool,
            tc.tile_pool(name="wt", bufs=1) as wtpool,
            tc.tile_pool(name="rec", bufs=8) as rec,
            tc.tile_pool(name="wog", bufs=6) as wog_pool,
            tc.tile_pool(name="xp0", bufs=2 * KT) as xp0,
            tc.tile_pool(name="xp", bufs=10) as xp,
            tc.tile_pool(name="op", bufs=8) as op,
            tc.tile_pool(name="psum", bufs=6, space="PSUM") as pp,
            tc.tile_pool(name="psum_r", bufs=2, space="PSUM") as ppr,
        ):
            # --- constants ---
            ident = cpool.tile([P, P], BF16, name="ident")
            make_identity(nc, ident)
            ones = cpool.tile([1, P], F32, name="ones")
            nc.vector.memset(ones, 1.0)
            bias_row = cpool.tile([1, O_SH], F32, name="bias_row")
            nc.sync.dma_start(out=bias_row, in_=bias_d.ap().unsqueeze(0))
            bias_bc = cpool.tile([P, O_SH], F32, name="bias_bc")
            pbt = ppr.tile([P, NFREE], F32, tag="psr", name="psb")
            nc.tensor.matmul(pbt, ones, bias_row, start=True, stop=True)
            nc.vector.tensor_copy(out=bias_bc, in_=pbt)

            # --- alpha (per-partition scalars), all o-tiles resident ---
            alpha_sb = []
            for ot in range(OT):
                at = cpool.tile([P, G, QB], F32, name=f"alpha{ot}")
                nc.scalar.dma_start(out=at, in_=alpha_d.ap()[ot * P : (ot + 1) * P])
                alpha_sb.append(at)

            # --- W^T shard, resident ---
            wt_tiles = [
                wtpool.tile([P, O_SH], BF16, tag=f"wt{k}", name=f"wt{k}")
                for k in range(KT)
            ]

            def recon_group(g, bts):
                """Reconstruct wt_tiles[g] from bit planes (bts = OT fp8
                tiles for this gp). ACT does plane-0 mul; DVE the two
                fused mul-adds; PE transposes into a dedicated recon bank;
                copies split ACT/DVE to balance."""
                gg = g % GMERGE
                ptt = ppr.tile([P, 2 * NFREE], BF16, tag="psr", name=f"ptr{g}")
                for ot in range(OT):
                    bt = bts[ot]
                    at = alpha_sb[ot]
                    w_og = wog_pool.tile([P, P], BF16, tag="wog")
                    nc.scalar.activation(
                        w_og,
                        bt[:, gg, :, 0],
                        mybir.ActivationFunctionType.Copy,
                        scale=at[:, g, 0:1],
                    )
                    nc.vector.scalar_tensor_tensor(
                        w_og, bt[:, gg, :, 1], at[:, g, 1:2], w_og, mult, add
                    )
                    nc.vector.scalar_tensor_tensor(
                        w_og, bt[:, gg, :, 2], at[:, g, 2:3], w_og, mult, add
                    )
                    nc.tensor.transpose(
                        ptt[:, ot * P : (ot + 1) * P], w_og, ident
                    )
                if g % 2 == 0:
                    nc.scalar.copy(out=wt_tiles[g], in_=ptt[:, 0:O_SH])
                else:
                    nc.vector.tensor_copy(out=wt_tiles[g], in_=ptt[:, 0:O_SH])

            def mm_unit(bank, xs, k):
                nc.tensor.matmul(
                    bank[:, 0:H], xs, wt_tiles[k][:, 0:H],
                    start=(k == 0), stop=False, skip_group_check=True,
                )
                nc.tensor.matmul(
                    bank[:, H:NFREE], xs, wt_tiles[k][:, H:NFREE],
                    start=False, stop=(k == KT - 1), skip_group_check=True,
                )

            def drain(bank, c, s):
                os_t = op.tile([P, NFREE], BF16, tag="os")
                nc.vector.tensor_tensor(out=os_t, in0=bank, in1=bias_bc, op=add)
                nc.sync.dma_start(
                    out=out_d.ap()[
                        c * BCHUNK + s * P : c * BCHUNK + (s + 1) * P, :
                    ],
                    in_=os_t,
                )

            # --- fused phase: recon + chunks 0/1 (chunk1 s0,s1 only) ---
            banks0 = [pp.tile([P, NFREE], F32, tag="ps", name=f"c0_{s}")
                      for s in range(NSUB)]
            banks1 = [pp.tile([P, NFREE], F32, tag="ps", name=f"c1_{s}")
                      for s in range(2)]
            xt1 = []  # chunk 1 x tiles, kept for the deferred s2/s3 pass
            bts = None
            for g in range(G):
                gp, gg = divmod(g, GMERGE)
                if gg == 0:
                    bts = []
                    for ot in range(OT):
                        bt = rec.tile([P, GMERGE, A, QB], FP8, tag="bt")
                        nc.sync.dma_start(
                            out=bt,
                            in_=binary_d.ap()[
                                ot * P : (ot + 1) * P,
                                gp * GMERGE : (gp + 1) * GMERGE,
                            ],
                        )
                        bts.append(bt)
                recon_group(g, bts)
                k = g
                xt_c0 = xp0.tile([P, BCHUNK], BF16, tag="x0")
                nc.sync.dma_start(out=xt_c0, in_=xt_d.ap()[:, k, 0:BCHUNK])
                xt_c1 = xp0.tile([P, BCHUNK], BF16, tag="x0")
                nc.scalar.dma_start(
                    out=xt_c1, in_=xt_d.ap()[:, k, BCHUNK : 2 * BCHUNK]
                )
                xt1.append(xt_c1)
                for s in range(NSUB):
                    mm_unit(banks0[s], xt_c0[:, s * P : (s + 1) * P], k)
                for s in range(2):
                    mm_unit(banks1[s], xt_c1[:, s * P : (s + 1) * P], k)

            for s in range(NSUB):
                drain(banks0[s], 0, s)

            # --- deferred: chunk 1, s2/s3 (x tiles still resident) ---
            banks1b = [pp.tile([P, NFREE], F32, tag="ps", name=f"c1b_{s}")
                       for s in range(2)]
            for k in range(KT):
                for i, s in enumerate((2, 3)):
                    mm_unit(banks1b[i], xt1[k][:, s * P : (s + 1) * P], k)
            for s in range(2):
                drain(banks1[s], 1, s)
            for i, s in enumerate((2, 3)):
                drain(banks1b[i], 1, s)

            # --- remaining chunks ---
            n_chunks = BS_SH // BCHUNK
            for c in range(2, n_chunks):
                banks = [pp.tile([P, NFREE], F32, tag="ps", name=f"mm{c}_{s}")
                         for s in range(NSUB)]
                for k in range(KT):
                    xt_t = xp.tile([P, BCHUNK], BF16, tag="xt")
                    dma_eng = nc.sync if k % 2 == 0 else nc.scalar
                    dma_eng.dma_start(
                        out=xt_t,
                        in_=xt_d.ap()[:, k, c * BCHUNK : (c + 1) * BCHUNK],
                    )
                    for s in range(NSUB):
                        mm_unit(banks[s], xt_t[:, s * P : (s + 1) * P], k)
                for s in range(NSUB):
                    drain(banks[s], c, s)

    if not nc.is_finalized():
        nc.finalize()
    return nc


def shard_inputs(x, alpha, bias, binary):
    """Host-side slicing/relayout/cast only. Returns per-core input maps."""
    x2 = np.ascontiguousarray(x).reshape(BS, I)
    xtp = np.ascontiguousarray(
        x2.T.reshape(KT, P, BS).transpose(1, 0, 2).astype(NP_BF16)
    )
    alpha = np.ascontiguousarray(alpha)
    binary = np.ascontiguousarray(binary).astype(NP_FP8)
    bias = np.ascontiguousarray(bias)

    in_maps = []
    for c in range(N_CORES):
        osl = slice(c * O_SH, (c + 1) * O_SH)
        in_maps.append(
            {
                "xt": xtp,
                "alpha": alpha[osl],
                "binary": binary[osl],
                "bias": bias[osl],
            }
        )
    return in_maps


def assemble_output(results):
    out = np.empty((BS, O), dtype=np.float32)
    for c in range(N_CORES):
        out[:, c * O_SH : (c + 1) * O_SH] = results[c]["out"].astype(np.float32)
    return out.reshape(B, S, O)


_NC_CACHE = None


def kernel(x, alpha, bias, binary):
    global _NC_CACHE
    if _NC_CACHE is None:
        _NC_CACHE = build_nc()
    nc = _NC_CACHE
    in_maps = shard_inputs(
        np.asarray(x, dtype=np.float32),
        np.asarray(alpha, dtype=np.float32),
        np.asarray(bias, dtype=np.float32),
        np.asarray(binary, dtype=np.float32),
    )
    res = run_bass_kernel_spmd(nc, in_maps, list(range(N_CORES)))
    return assemble_output(res.results)
